# revision 1
# baseline (speedup 1.0000x reference)
"""Canny edge detector on 8 Trainium2 NeuronCores (Bass/Tile).

Sharding: row slabs. Core i owns output rows [118*i, 118*(i+1)) of ALL 8
images. (The reference's flat gather at B=8 cross-wires images inside NMS:
sel_pos(b,h,w) = dirconv_b(gm_{idx(b,h,w)})(h,w), so every output pixel needs
all 8 images' gradient-magnitude maps at its rows -> shard by rows, not by
image.) The leftover band (rows 944..1023) is computed per-image on the
owning core and the gm maps are exchanged through DRAM collectives
(AllGather for plain maps, AllToAll for reader-shift-specific maps).

All compute-engine APs must start at partition 0 (HW constraint), so row
re-alignment between pipeline stages is done with SBUF->SBUF DMAs.
"""

import os

# Tile's subtile dependency tracking emits >1 embedded sync-wait on
# S2S2D2_STT instructions, which the ISA encoding cannot hold ("Too many
# sync wait commands" in codegen). With whole-tile deps the wait-absorbing
# guard ops keep every STT at <=1 embedded wait.
os.environ.setdefault("BY_DEFAULT_DISABLE_SUBTILE_DEPS", "1")

import numpy as np

H = 1024
W = 1024
B = 8
NC = 8
SLAB = 118                    # main-slab output rows per core
B8_START = SLAB * NC          # 944
B8_ROWS = H - B8_START        # 80
LOW_T, HIGH_T = 2.5, 5.0
T22SQ = float(np.float32(np.tan(np.pi / 8.0)) ** 2)

# direction index -> (dr, dc) neighbor offset of dir_f channel d
DELTAS = {0: (0, 1), 1: (1, 1), 2: (1, 0), 3: (1, -1),
          4: (0, -1), 5: (-1, -1), 6: (-1, 0), 7: (-1, 1)}


def _gauss5():
    n = np.arange(5, dtype=np.float32) - 2.0
    return np.exp(-0.5 * n * n).astype(np.float32)


def _band(n_in, n_out, offset, taps):
    """M[k, m] = taps[k - m - offset] for k-m-offset in range(len(taps))."""
    m_ = np.zeros((n_in, n_out), np.float32)
    for mm in range(n_out):
        for t, w in enumerate(taps):
            k = mm + offset + t
            if 0 <= k < n_in:
                m_[k, mm] = w
    return m_


def _const_mats(core):
    g = _gauss5()
    g0 = float(g[0])
    mats = {}
    # main slab: x/hb tile row k <-> img row a+k, a = 118i-5
    # bl row m <-> img a+2+m (124 rows); BV[k,m] = g0*g[k-m]
    mats["BV"] = _band(128, 124, 0, (g0 * g).tolist())
    # gx/gy row m <-> img a+3+m = 118i-2+m (122 rows); bl k <-> a+2+k
    b121 = _band(124, 122, 0, [1.0, 2.0, 1.0])
    b10m1 = _band(124, 122, 0, [1.0, 0.0, -1.0])
    if core == 0:  # img rows -2,-1 must yield gm=0 (zero-pad semantics)
        b121[:, 0:2] = 0.0
        b10m1[:, 0:2] = 0.0
    mats["B121"] = b121
    mats["B121N"] = -b121
    mats["B10M1"] = b10m1
    mats["B10M1X2"] = 2.0 * b10m1
    # strong row k (base 0) <-> thin img row 118i-1+k
    # mp row p <-> img 118i-1+p (col 0 dummy); needs strong k = p-1,p,p+1
    bones = _band(120, 119, -1, [1.0, 1.0, 1.0])
    bones[:, 0] = 0.0
    if core == 0:
        bones[:, 1] = 0.0  # border row 0
    mats["BONES"] = bones
    # B8 block: x8 row k <-> img 936+k; bl8 row m <-> img 938+m (86 rows)
    mats["BV8"] = _band(88, 86, 0, (g0 * g).tolist())
    # gx8 row m <-> img 940+m (84 rows); bl8 k <-> 938+k: band k-m in {1,2,3}
    b121_8 = _band(86, 84, 1, [1.0, 2.0, 1.0])
    b10m1_8 = _band(86, 84, 1, [1.0, 0.0, -1.0])
    mats["B121_8"] = b121_8
    mats["B121N_8"] = -b121_8
    mats["B10M1_8"] = b10m1_8
    mats["B10M1X2_8"] = 2.0 * b10m1_8
    # strong8 row k (base 0) <-> img 943+k; mp8 row p <-> img 943+p
    # (col 0 dummy); needs strong8 k = p-1,p,p+1
    bones8 = _band(81, 81, -1, [1.0, 1.0, 1.0])
    bones8[:, 0] = 0.0
    bones8[:, 80] = 0.0  # border row 1023
    mats["BONES8"] = bones8
    return {k: np.ascontiguousarray(v, np.float32) for k, v in mats.items()}


MAT_SPECS = {
    "BV": [128, 124], "B121": [124, 122], "B121N": [124, 122],
    "B10M1": [124, 122], "B10M1X2": [124, 122], "BONES": [120, 119],
    "BV8": [88, 86], "B121_8": [86, 84], "B121N_8": [86, 84],
    "B10M1_8": [86, 84], "B10M1X2_8": [86, 84], "BONES8": [81, 81],
}

_CACHE = {}


def _build_program():
    if "nc" in _CACHE:
        return _CACHE["nc"]
    import concourse.bass as bass
    import concourse.mybir as mybir
    from concourse.tile import TileContext

    f32 = mybir.dt.float32
    bf16 = mybir.dt.bfloat16
    u8 = mybir.dt.uint8
    Alu = mybir.AluOpType

    g = _gauss5()
    r01 = float(g[0] / g[1])
    r12 = float(g[1] / g[2])
    r21 = float(g[2] / g[1])
    r10 = float(g[1] / g[0])

    nc = bass.Bass()

    def guard(out_ap, in0_ap, in1_ap):
        # Obsolete: _legalize_waits() NoOp-splits any multi-wait instruction
        # after scheduling, which is cheaper than extra DVE data ops.
        pass

    def fence(t):
        pass

    xm = nc.declare_dram_parameter("xm", [B * 3, 128, W], f32, isOutput=False)
    x8 = nc.declare_dram_parameter("x8", [3, 88, W], f32, isOutput=False)
    mat_d = {k: nc.declare_dram_parameter(k, v, f32, isOutput=False)
             for k, v in MAT_SPECS.items()}
    outm = nc.declare_dram_parameter("outm", [B, SLAB, W], f32, isOutput=True)
    out8 = nc.declare_dram_parameter("out8", [B8_ROWS, W], f32, isOutput=True)

    with TileContext(nc) as tc:
        with (
            tc.tile_pool(name="consts", bufs=1) as cpool,
            tc.tile_pool(name="gmp", bufs=1) as gmpool,
            tc.tile_pool(name="msk", bufs=1) as mskpool,
            tc.tile_pool(name="dram", bufs=1, space="DRAM") as dpool,
        ):
            mt = {}
            for name, shp in MAT_SPECS.items():
                t = cpool.tile(shp, f32, tag=name)
                nc.sync.dma_start(out=t[:], in_=mat_d[name][:])
                mt[name] = t

            gm_tiles = []
            masks = []
            # =========== conv phase (own scoped pools) =====================
            with (
                tc.tile_pool(name="xin", bufs=3) as xpool,
                tc.tile_pool(name="hbt", bufs=2) as hbpool,
                tc.tile_pool(name="bls", bufs=2) as blspool,
                tc.tile_pool(name="sq", bufs=2) as sqpool,
                tc.tile_pool(name="gsum", bufs=2) as gsumpool,
                tc.tile_pool(name="mskt", bufs=2) as msktpool,
                tc.tile_pool(name="psA", bufs=2, space="PSUM") as psA,
                tc.tile_pool(name="psB", bufs=1, space="PSUM") as psB,
            ):
                def conv_pipeline(xt, n_in, bv, b121, b121n, b10m1, b10m1x2,
                                  gm_acc, gxs, gys, c, n_bl, n_gxy,
                                  pe_hblur=False):
                    bl = psA.tile([n_bl, W], f32, tag="bl")
                    if pe_hblur:
                        # full 2D blur as 5 shifted-column accumulated
                        # streams: bl = sum_h (g_h * band(g)) @ x<<(h-2)
                        lhs5 = [bv, mt["BVG1"], mt["BVG2"], mt["BVG1"], bv]
                        for lo in (0, 512):
                            for h in range(5):
                                nc.tensor.matmul(
                                    out=bl[:, lo:lo + 512],
                                    lhsT=lhs5[h][0:n_in, 0:n_bl],
                                    rhs=xt[:, h + lo:h + lo + 512],
                                    start=(h == 0), stop=(h == 4))
                    else:
                        # H-blur (Horner, 4 fused ops) -> h2 [n_in, W]
                        h1 = hbpool.tile([n_in, W], f32, tag="h1")
                        h2 = hbpool.tile([n_in, W], f32, tag="h2")
                        guard(h1, xt, h2)
                        guard(h2, xt, h1)
                        nc.vector.scalar_tensor_tensor(
                            out=h1[:], in0=xt[:, 0:W], scalar=r01,
                            in1=xt[:, 1:W + 1], op0=Alu.mult, op1=Alu.add)
                        nc.vector.scalar_tensor_tensor(
                            out=h2[:], in0=h1[:], scalar=r12,
                            in1=xt[:, 2:W + 2], op0=Alu.mult, op1=Alu.add)
                        nc.vector.scalar_tensor_tensor(
                            out=h1[:], in0=h2[:], scalar=r21,
                            in1=xt[:, 3:W + 3], op0=Alu.mult, op1=Alu.add)
                        nc.vector.scalar_tensor_tensor(
                            out=h2[:], in0=h1[:], scalar=r10,
                            in1=xt[:, 4:W + 4], op0=Alu.mult, op1=Alu.add)
                        for lo in (0, 512):
                            nc.tensor.matmul(out=bl[:, lo:lo + 512],
                                             lhsT=bv[0:n_in, 0:n_bl],
                                             rhs=h2[:, lo:lo + 512],
                                             start=True, stop=True)
                    # copy to SBUF with 1-col zero margins
                    blt = blspool.tile([n_bl, W + 2], f32, tag="bls")
                    fence(blt)
                    nc.vector.memset(blt[:, 0:1], 0.0)
                    nc.vector.memset(blt[:, W + 1:W + 2], 0.0)
                    nc.scalar.copy(out=blt[:, 1:W + 1], in_=bl[:])
                    blm = blt[:, 0:W]
                    blc = blt[:, 1:W + 1]
                    blp = blt[:, 2:W + 2]
                    # sobel on PE: gx = B121@blm - B121@blp
                    #              gy = B10M1@(blp+blm) + 2*B10M1@blc
                    gx = psB.tile([n_gxy, W], f32, tag="gx")
                    gy = psB.tile([n_gxy, W], f32, tag="gy")
                    for lo in (0, 512):
                        nc.tensor.matmul(out=gx[:, lo:lo + 512],
                                         lhsT=b121[0:n_bl, 0:n_gxy],
                                         rhs=blm[:, lo:lo + 512],
                                         start=True, stop=False)
                        nc.tensor.matmul(out=gx[:, lo:lo + 512],
                                         lhsT=b121n[0:n_bl, 0:n_gxy],
                                         rhs=blp[:, lo:lo + 512],
                                         start=False, stop=True)
                        nc.tensor.matmul(out=gy[:, lo:lo + 512],
                                         lhsT=b10m1[0:n_bl, 0:n_gxy],
                                         rhs=blp[:, lo:lo + 512],
                                         start=True, stop=False)
                        nc.tensor.matmul(out=gy[:, lo:lo + 512],
                                         lhsT=b10m1x2[0:n_bl, 0:n_gxy],
                                         rhs=blc[:, lo:lo + 512],
                                         start=False, stop=False)
                        nc.tensor.matmul(out=gy[:, lo:lo + 512],
                                         lhsT=b10m1[0:n_bl, 0:n_gxy],
                                         rhs=blm[:, lo:lo + 512],
                                         start=False, stop=True)
                    # magnitude
                    sqx = sqpool.tile([n_gxy, W], f32, tag="sqx")
                    sqy = sqpool.tile([n_gxy, W], f32, tag="sqy")
                    nc.scalar.square(out=sqx[:], in_=gx[:])
                    nc.scalar.square(out=sqy[:], in_=gy[:])
                    m2 = sqpool.tile([n_gxy, W], f32, tag="m2")
                    nc.gpsimd.tensor_tensor(out=m2[:], in0=sqx[:], in1=sqy[:],
                                            op=Alu.add)
                    if c == 0:
                        nc.scalar.sqrt(out=gm_acc[0:n_gxy, 1:W + 1], in_=m2[:])
                    else:
                        magt = sqpool.tile([n_gxy, W], f32, tag="magt")
                        nc.scalar.sqrt(out=magt[:], in_=m2[:])
                        nc.gpsimd.tensor_tensor(
                            out=gm_acc[0:n_gxy, 1:W + 1],
                            in0=gm_acc[0:n_gxy, 1:W + 1],
                            in1=magt[:], op=Alu.add)
                    # gxs/gys accumulation (full range, base partition 0)
                    if c == 0:
                        nc.scalar.copy(out=gxs[0:n_gxy, :], in_=gx[:])
                        nc.scalar.copy(out=gys[0:n_gxy, :], in_=gy[:])
                    else:
                        nc.vector.tensor_tensor(out=gxs[0:n_gxy, :],
                                                in0=gxs[0:n_gxy, :],
                                                in1=gx[:], op=Alu.add)
                        nc.vector.tensor_tensor(out=gys[0:n_gxy, :],
                                                in0=gys[0:n_gxy, :],
                                                in1=gy[:], op=Alu.add)

                def make_masks(gxs, gys, n, shift, n_thin, j):
                    """u8 masks computed at conv frame [0:n], DMA-shifted down
                    by `shift` rows into persistent thin-frame tiles."""
                    a2 = sqpool.tile([n, W], f32, tag="sqx")
                    b2 = sqpool.tile([n, W], f32, tag="sqy")
                    nc.scalar.square(out=a2[:, :], in_=gxs[0:n, :])
                    nc.scalar.square(out=b2[:, :], in_=gys[0:n, :])
                    tmp = [msktpool.tile([n, W], u8, tag=t, name=t)
                           for t in ("tc0", "tc2", "tsm")]
                    guard(tmp[0], a2, b2)
                    guard(tmp[1], a2, b2)
                    nc.vector.scalar_tensor_tensor(
                        out=tmp[0][:], in0=a2[:], scalar=T22SQ,
                        in1=b2[:], op0=Alu.mult, op1=Alu.is_gt)
                    nc.vector.scalar_tensor_tensor(
                        out=tmp[1][:], in0=b2[:], scalar=T22SQ,
                        in1=a2[:], op0=Alu.mult, op1=Alu.is_gt)
                    ab = sqpool.tile([n, W], f32, tag="m2")
                    nc.gpsimd.tensor_tensor(out=ab[:], in0=gxs[0:n, :],
                                            in1=gys[0:n, :], op=Alu.mult)
                    guard(tmp[2], ab, ab)
                    nc.vector.tensor_scalar(out=tmp[2][:], in0=ab[:],
                                            scalar1=0.0, scalar2=None,
                                            op0=Alu.is_ge)
                    out = []
                    for t, tag in zip(tmp, ("c0", "c2", "sm")):
                        p = mskpool.tile([n_thin, W], u8, tag=f"{tag}_{j}")
                        fence(p)
                        nc.sync.dma_start(out=p[:],
                                          in_=t[shift:shift + n_thin, :])
                        out.append(p)
                    return out

                # main slab: 8 images x 3 channels
                for j in range(B):
                    gm_j = gmpool.tile([122, W + 2], f32, tag=f"gm{j}")
                    nc.vector.memset(gm_j[:, 0:1], 0.0)
                    nc.vector.memset(gm_j[:, W + 1:W + 2], 0.0)
                    gxs = gsumpool.tile([122, W], f32, tag="gxs")
                    gys = gsumpool.tile([122, W], f32, tag="gys")
                    for c in range(3):
                        xt = xpool.tile([128, W + 4], f32, tag="x")
                        fence(xt)
                        nc.vector.memset(xt[:, 0:2], 0.0)
                        nc.vector.memset(xt[:, W + 2:W + 4], 0.0)
                        nc.sync.dma_start(out=xt[:, 2:W + 2], in_=xm[3 * j + c])
                        conv_pipeline(xt, 128, mt["BV"], mt["B121"],
                                      mt["B121N"], mt["B10M1"], mt["B10M1X2"],
                                      gm_j, gxs, gys, c, 124, 122)
                    gm_tiles.append(gm_j)
                    # thin frame = conv rows 1..120 -> shift 1, 120 rows
                    masks.append(make_masks(gxs, gys, 122, 1, 120, j))

                # B8 block (own image); gm8 row p <-> img 940+p, row 84 = 0
                gm8 = gmpool.tile([85, W + 2], f32, tag="gm8self")
                nc.vector.memset(gm8[:], 0.0)
                gxs8 = gsumpool.tile([84, W], f32, tag="gxs")
                gys8 = gsumpool.tile([84, W], f32, tag="gys")
                for c in range(3):
                    xt = xpool.tile([88, W + 4], f32, tag="x")
                    fence(xt)
                    nc.vector.memset(xt[:, 0:2], 0.0)
                    nc.vector.memset(xt[:, W + 2:W + 4], 0.0)
                    nc.sync.dma_start(out=xt[:, 2:W + 2], in_=x8[c])
                    conv_pipeline(xt, 88, mt["BV8"], mt["B121_8"],
                                  mt["B121N_8"], mt["B10M1_8"],
                                  mt["B10M1X2_8"], gm8, gxs8, gys8, c, 86, 84)
                # thin8 frame = conv rows 3..83 -> shift 3, 81 rows
                m8 = make_masks(gxs8, gys8, 84, 3, 81, 8)

            # =========== B8 gm exchange ===================================
            ag_in = dpool.tile([81, W], f32, tag="ag_in")
            ag_out = dpool.tile([B * 81, W], f32, tag="ag_out")
            fence(gm8)
            nc.sync.dma_start(out=ag_in[:], in_=gm8[3:84, 1:W + 1])
            nc.gpsimd.collective_compute(
                "AllGather", Alu.bypass, replica_groups=[list(range(NC))],
                ins=[ag_in.opt()], outs=[ag_out.opt()])
            a2a_in = dpool.tile([B * 81, W], f32, tag="a2a_in")
            a2a_out = dpool.tile([B * 81, W], f32, tag="a2a_out")
            for b in range(B):
                dr, dc = DELTAS[b]
                nc.sync.dma_start(
                    out=a2a_in[81 * b:81 * (b + 1), :],
                    in_=gm8[3 + dr:84 + dr, 1 + dc:W + 1 + dc])
            nc.gpsimd.collective_compute(
                "AllToAll", Alu.bypass, replica_groups=[list(range(NC))],
                ins=[a2a_in.opt()], outs=[a2a_out.opt()])

            # =========== NMS phase (own scoped pools) ======================
            # thin frame: row p (base 0) <-> img row 118i-1+p, 120 rows.
            with (
                tc.tile_pool(name="ce", bufs=1) as cepool,
                tc.tile_pool(name="shp", bufs=1) as shpool,
                tc.tile_pool(name="cmap", bufs=2) as cpool2,
                tc.tile_pool(name="g8p", bufs=2) as g8pool,
                tc.tile_pool(name="pmap", bufs=1) as ppool,
                tc.tile_pool(name="nmst", bufs=1) as npool,
                tc.tile_pool(name="outp", bufs=2) as opool,
                tc.tile_pool(name="psC", bufs=2, space="PSUM") as psC,
            ):
                # center-aligned copies of gm (thin frame)
                ce = []
                for j in range(B):
                    fence(gm_tiles[j])
                    t = cepool.tile([120, W + 2], f32, tag=f"ce{j}")
                    nc.sync.dma_start(out=t[:], in_=gm_tiles[j][1:121, :])
                    ce.append(t)

                def build_shift(drow):
                    tiles = []
                    for j in range(B):
                        t = shpool.tile([120, W + 2], f32, tag=f"sh{j}")
                        fence(t)
                        if drow == 1:
                            nc.sync.dma_start(out=t[:],
                                              in_=gm_tiles[j][2:122, :])
                        else:
                            nc.sync.dma_start(out=t[:],
                                              in_=gm_tiles[j][0:120, :])
                        tiles.append(t)
                    return tiles

                def nms_core(b_masks, gm_b, get_in0, get_in1, n_thin,
                             bones, n_mp, out_lo, out_dram, n_out):
                    """Shared NMS tail; all tiles base partition 0."""
                    c0, c2, sm = b_masks
                    P = []
                    for k in range(4):
                        Cs = []
                        for j in (k, k + 4):
                            cj = cpool2.tile([n_thin, W], bf16, tag="c")
                            nc.vector.tensor_tensor(out=cj[:], in0=get_in0(j),
                                                    in1=get_in1(j),
                                                    op=Alu.is_gt)
                            Cs.append(cj)
                        tag = "psel" if k == 3 else f"p{k}"
                        bufs_k = 2 if k == 3 else None
                        pk = ppool.tile([n_thin, W], bf16, tag=tag,
                                        bufs=bufs_k)
                        nc.vector.tensor_tensor(out=pk[:], in0=Cs[0][:],
                                                in1=Cs[1][:],
                                                op=Alu.logical_and)
                        P.append(pk)
                    psel = P[3]
                    nc.vector.copy_predicated(out=psel[:], mask=sm[:],
                                              data=P[1][:])
                    nc.vector.copy_predicated(out=psel[:], mask=c0[:],
                                              data=P[0][:])
                    nc.vector.copy_predicated(out=psel[:], mask=c2[:],
                                              data=P[2][:])
                    strong = npool.tile([n_thin, W + 2], f32, tag="strong", bufs=2)
                    fence(strong)
                    nc.vector.memset(strong[:, 0:1], 0.0)
                    nc.vector.memset(strong[:, W + 1:W + 2], 0.0)
                    guard(strong, gm_b, psel)
                    nc.vector.scalar_tensor_tensor(
                        out=strong[:, 1:W + 1], in0=gm_b, scalar=HIGH_T,
                        in1=psel[:], op0=Alu.is_gt, op1=Alu.logical_and)
                    q = npool.tile([n_thin, W], f32, tag="q")
                    guard(q, gm_b, psel)
                    nc.vector.scalar_tensor_tensor(
                        out=q[:], in0=gm_b, scalar=LOW_T, in1=psel[:],
                        op0=Alu.is_ge, op1=Alu.logical_and)
                    mh = npool.tile([n_thin, W], f32, tag="mh")
                    nc.gpsimd.tensor_tensor(out=mh[:], in0=strong[:, 0:W],
                                            in1=strong[:, 2:W + 2],
                                            op=Alu.add)
                    nc.gpsimd.tensor_tensor(out=mh[:], in0=mh[:],
                                            in1=strong[:, 1:W + 1],
                                            op=Alu.add)
                    mp = psC.tile([n_mp, W], f32, tag="mp")
                    for lo2 in (0, 512):
                        nc.tensor.matmul(out=mp[:, lo2:lo2 + 512],
                                         lhsT=bones[0:n_thin, 0:n_mp],
                                         rhs=mh[:, lo2:lo2 + 512],
                                         start=True, stop=True)
                    ot = opool.tile([n_mp, W], f32, tag="ot")
                    guard(ot, mp, q)
                    nc.vector.scalar_tensor_tensor(
                        out=ot[:], in0=mp[:], scalar=0.5, in1=q[0:n_mp, :],
                        op0=Alu.is_ge, op1=Alu.logical_and)
                    nc.vector.memset(ot[:, 0:1], 0.0)
                    nc.vector.memset(ot[:, W - 1:W], 0.0)
                    nc.sync.dma_start(out=out_dram,
                                      in_=ot[out_lo:out_lo + n_out, :])

                def nms_b(b, shifted):
                    dr, dc = DELTAS[b]

                    def in0(j):
                        return ce[j][:, 1:W + 1]

                    def in1(j):
                        src = ce[j] if dr == 0 else shifted[j]
                        return src[:, 1 + dc:W + 1 + dc]

                    nms_core(masks[b], ce[b][:, 1:W + 1], in0, in1, 120,
                             mt["BONES"], 119, 1, outm[b], SLAB)

                for b in (0, 4):
                    nms_b(b, None)
                dn = build_shift(1)
                for b in (1, 2, 3):
                    nms_b(b, dn)
                up = build_shift(-1)
                for b in (5, 6, 7):
                    nms_b(b, up)

                # B8: own image only; operands pre-shifted via AllToAll.
                # thin8 frame: row p (base 0) <-> img 943+p, 81 rows.
                ce8 = g8pool.tile([81, W], f32, tag="ce8", bufs=1)
                nc.sync.dma_start(out=ce8[:], in_=gm8[3:84, 1:W + 1])

                def load8(dram_src, tag):
                    def get(j):
                        t = g8pool.tile([81, W], f32, tag=tag)
                        fence(t)
                        nc.sync.dma_start(
                            out=t[:], in_=dram_src[81 * j:81 * (j + 1), :])
                        return t[:]
                    return get

                nms_core(m8, ce8[:], load8(ag_out, "gp8"),
                         load8(a2a_out, "gs8"), 81,
                         mt["BONES8"], 81, 1, out8[:], B8_ROWS)

    _legalize_waits(nc)
    _CACHE["nc"] = nc
    return nc


def _legalize_waits(nc):
    """Several ISA encodings (S2S2D2_STT, HWDGE DMACopy, ...) hold only one
    embedded sync-wait, but Tile's scheduler can attach more. Hoist all
    embedded waits of multi-wait instructions into a NoOp injected just
    before them on the same engine queue (NoOps carry many waits fine)."""
    import concourse.mybir as mybir
    n = 0
    for f in nc.m.functions:
        for blk in f.blocks:
            out = []
            for ins in blk.instructions:
                si = ins.sync_info
                if (si is not None and si.on_wait is not None
                        and len(si.on_wait) > 1):
                    for w in si.on_wait:
                        nop = mybir.InstNoOp(
                            name=f"WFIX-{n}", engine=ins.engine,
                            sync_info=mybir.SyncInfo(on_wait=[w],
                                                     on_update=[]))
                        n += 1
                        out.append(nop)
                    ins.sync_info = mybir.SyncInfo(
                        on_wait=[],
                        on_update=list(si.on_update or []))
                out.append(ins)
            blk.instructions = out


def _in_maps(img):
    img = np.ascontiguousarray(img, dtype=np.float32)
    pad = np.zeros((B, 3, 5, W), np.float32)
    imgp = np.concatenate([pad, img], axis=2)  # rows shifted by +5
    maps = []
    for i in range(NC):
        r0 = SLAB * i  # padded row index of img row 118i-5
        xm_i = imgp[:, :, r0:r0 + 128, :].reshape(B * 3, 128, W)
        x8_i = img[i, :, B8_START - 8:, :]  # img rows 936..1023
        m = {"xm": np.ascontiguousarray(xm_i),
             "x8": np.ascontiguousarray(x8_i)}
        m.update(_const_mats(i))
        maps.append(m)
    return maps


def kernel(img, gauss_h=None, gauss_v=None, sobel_h=None, sobel_v=None,
           dir_f=None, connect_f=None, _want_time=False):
    from concourse.bass_utils import run_bass_kernel_spmd
    nc = _build_program()
    maps = _in_maps(np.asarray(img))
    res = run_bass_kernel_spmd(nc, maps, list(range(NC)), trace=_want_time)
    out = np.zeros((B, 1, H, W), np.float32)
    for i in range(NC):
        r = res.results[i]
        out[:, 0, SLAB * i:SLAB * (i + 1), :] = r["outm"]
        out[i, 0, B8_START:, :] = r["out8"]
    if _want_time:
        return out, res
    return out



# revision 13
# speedup vs baseline: 1.2546x; 1.2546x over previous
"""Canny edge detector on 8 Trainium2 NeuronCores (Bass/Tile).

Sharding: row slabs. Core i owns output rows [118*i, 118*(i+1)) of ALL 8
images. (The reference's flat gather at B=8 cross-wires images inside NMS:
sel_pos(b,h,w) = dirconv_b(gm_{idx(b,h,w)})(h,w), so every output pixel needs
all 8 images' gradient-magnitude maps at its rows -> shard by rows, not by
image.) The leftover band (rows 944..1023) is computed per-image on the
owning core; gm maps are exchanged through DRAM collectives (AllGather for
plain maps, AllToAll for reader-direction-shifted maps - the per-core shift
must live in data routing because SPMD shares one instruction stream). The
B8 block runs FIRST so both collectives hide under the main-slab conv.

Numerics: the output is a thresholded argmax - iid noise of eps relative
flips ~45k*eps pixels, and the gate (rel 2e-2 ~ 840 flips) needs eps <=
1.5e-5. fp16 or float32r matmuls are far too lossy (measured), BUT fp16
products accumulate exactly into fp32 PSUM, so the conv phase uses
fp16 PAIRS: x = x_hi + x_lo (host-split), bl = sum_h W_h @ {x_hi,x_lo};
blt = blt_hi + blt_lo (Act copy + DVE residual); sobel = exact +-1/2
fp16 weights on both halves. Only systematic error left is the fp16
rounding of the gauss weights, minimized by a host-side scale search
(the scale folds into the LOW/HIGH thresholds; masks are scale-free).
gm / orientation stay fp32; NMS indicator algebra runs in fp16/u8.

SBUF: directions 0-4 run the NMS in a row-shifted frame (tile row r =
thin row p + 1) so their shifted compare operand is the RAW gm tile and
only one aligned copy (ce = gm rows 1..121) is ever made - no dn tiles.

All compute-engine APs must start at partition 0 (HW constraint), so row
re-alignment between pipeline stages is done with SBUF->SBUF DMAs.
"""

import os

os.environ.setdefault("BY_DEFAULT_DISABLE_SUBTILE_DEPS", "1")

import numpy as np

H = 1024
W = 1024
B = 8
NC = 8
SLAB = 118                    # main-slab output rows per core
B8_START = SLAB * NC          # 944
B8_ROWS = H - B8_START        # 80
LOW_T, HIGH_T = 2.5, 5.0
T22SQ = float(np.float32(np.tan(np.pi / 8.0)) ** 2)

DELTAS = {0: (0, 1), 1: (1, 1), 2: (1, 0), 3: (1, -1),
          4: (0, -1), 5: (-1, -1), 6: (-1, 0), 7: (-1, 1)}


def _gauss5():
    n = np.arange(5, dtype=np.float32) - 2.0
    return np.exp(-0.5 * n * n).astype(np.float32)


def _wscale():
    """Scale s minimizing fp16 rounding of the 6 distinct 2D-gauss weights
    s*gi*gj. gm scales by s; the LOW/HIGH thresholds absorb it."""
    if "s" in _CACHE:
        return _CACHE["s"]
    g = _gauss5().astype(np.float64)
    prods = np.array([g[i] * g[j] for i in range(3) for j in range(i, 3)])
    best, bs = 1e9, 1.0
    for s in np.linspace(1.0, 2.0, 65536, endpoint=False):
        r = np.abs(np.float64(np.float16(s * prods)) / (s * prods) - 1.0)
        m = r.max()
        if m < best:
            best, bs = m, float(s)
    _CACHE["s"] = bs
    return bs


def _band(n_in, n_out, offset, taps):
    """M[k, m] = taps[k - m - offset] for k-m-offset in range(len(taps))."""
    m_ = np.zeros((n_in, n_out), np.float32)
    for mm in range(n_out):
        for t, w in enumerate(taps):
            k = mm + offset + t
            if 0 <= k < n_in:
                m_[k, mm] = w
    return m_


def _const_mats(core):
    g = _gauss5()
    s = _wscale()
    sg = [float(np.float16(s * g[0] * gv)) for gv in g]  # pre-rounded row h=0
    mats = {}
    # fused 2D blur: bl = sum_h band(f16(s*g_h*g)) @ x_colshift_h
    for name, gh in (("BV", g[0]), ("BVG1", g[1]), ("BVG2", g[2])):
        taps = [float(np.float16(s * gh * gv)) for gv in g]
        mats[name] = _band(128, 124, 0, taps)
        mats[name + "_8"] = _band(88, 86, 0, taps)
    b121 = _band(124, 122, 0, [1.0, 2.0, 1.0])
    b10m1 = _band(124, 122, 0, [1.0, 0.0, -1.0])
    if core == 0:  # img rows -2,-1 must yield gm=0 (zero-pad semantics)
        b121[:, 0:2] = 0.0
        b10m1[:, 0:2] = 0.0
    mats["B121"] = b121
    mats["B121N"] = -b121
    mats["B10M1"] = b10m1
    mats["B10M1X2"] = 2.0 * b10m1
    # thin frame (dirs 5-7): strong row k <-> thin p=k; mp row p; col0 dummy
    bones = _band(120, 119, -1, [1.0, 1.0, 1.0])
    bones[:, 0] = 0.0
    if core == 0:
        bones[:, 1] = 0.0  # border img row 0
    mats["BONES"] = bones
    # shifted frame (dirs 0-4): strong row k <-> thin p=k-1; mp row m <->
    # thin p=m-1; needs strong k in {m-1,m,m+1}
    bones_d = _band(121, 120, -1, [1.0, 1.0, 1.0])
    bones_d[:, 0:2] = 0.0
    if core == 0:
        bones_d[:, 2] = 0.0  # border img row 0 (thin p=1 -> m=2)
    mats["BONES_D"] = bones_d
    # B8 block: x8 row k <-> img 936+k; bl8 row m <-> img 938+m (86 rows)
    b121_8 = _band(86, 84, 1, [1.0, 2.0, 1.0])
    b10m1_8 = _band(86, 84, 1, [1.0, 0.0, -1.0])
    mats["B121_8"] = b121_8
    mats["B121N_8"] = -b121_8
    mats["B10M1_8"] = b10m1_8
    mats["B10M1X2_8"] = 2.0 * b10m1_8
    bones8 = _band(81, 81, -1, [1.0, 1.0, 1.0])
    bones8[:, 0] = 0.0
    bones8[:, 80] = 0.0  # border row 1023
    mats["BONES8"] = bones8
    return {k: np.ascontiguousarray(v, np.float16) for k, v in mats.items()}


MAT_SPECS = {
    "BV": [128, 124], "BVG1": [128, 124], "BVG2": [128, 124],
    "B121": [124, 122], "B121N": [124, 122],
    "B10M1": [124, 122], "B10M1X2": [124, 122],
    "BONES": [120, 119], "BONES_D": [121, 120],
    "BV_8": [88, 86], "BVG1_8": [88, 86], "BVG2_8": [88, 86],
    "B121_8": [86, 84], "B121N_8": [86, 84],
    "B10M1_8": [86, 84], "B10M1X2_8": [86, 84], "BONES8": [81, 81],
}

_CACHE = {}


def _build_program():
    if "nc" in _CACHE:
        return _CACHE["nc"]
    import concourse.bass as bass
    import concourse.mybir as mybir
    from concourse.tile import TileContext

    f32 = mybir.dt.float32
    f16 = mybir.dt.float16
    u8 = mybir.dt.uint8
    Alu = mybir.AluOpType
    s = _wscale()
    LOW_S, HIGH_S = LOW_T * s, HIGH_T * s

    nc = bass.Bass()

    xh = nc.declare_dram_parameter("xh", [B * 3, 128, W], f16, isOutput=False)
    xl = nc.declare_dram_parameter("xl", [B * 3, 128, W], f16, isOutput=False)
    x8h = nc.declare_dram_parameter("x8h", [3, 88, W], f16, isOutput=False)
    x8l = nc.declare_dram_parameter("x8l", [3, 88, W], f16, isOutput=False)
    mat_d = {k: nc.declare_dram_parameter(k, v, f16, isOutput=False)
             for k, v in MAT_SPECS.items()}
    outm = nc.declare_dram_parameter("outm", [B, SLAB, W], f16, isOutput=True)
    out8 = nc.declare_dram_parameter("out8", [B8_ROWS, W], f16, isOutput=True)

    with TileContext(nc) as tc:
        with (
            tc.tile_pool(name="consts", bufs=1) as cpool,
            tc.tile_pool(name="gmp", bufs=1) as gmpool,
            tc.tile_pool(name="msk", bufs=1) as mskpool,
            tc.tile_pool(name="dram", bufs=1, space="DRAM") as dpool,
        ):
            mt = {}
            for name, shp in MAT_SPECS.items():
                t = cpool.tile(shp, f16, tag=name, name=name)
                nc.sync.dma_start(out=t[:], in_=mat_d[name][:])
                mt[name] = t

            gm_tiles = []
            masks = []
            ce = []
            gm8 = gmpool.tile([85, W + 2], f32, tag="gm8self")
            ce8 = gmpool.tile([81, W], f32, tag="ce8self")
            # =========== conv phase =======================================
            with (
                tc.tile_pool(name="xin", bufs=3) as xpool,
                tc.tile_pool(name="bls", bufs=2) as blspool,
                tc.tile_pool(name="sq", bufs=1) as sqpool,
                tc.tile_pool(name="gsm", bufs=2) as gspool,
                tc.tile_pool(name="mskt", bufs=2) as msktpool,
                tc.tile_pool(name="psA", bufs=2, space="PSUM") as psA,
                tc.tile_pool(name="psB", bufs=2, space="PSUM") as psB,
            ):
                def conv_pipeline(c, n_in, n_bl, n_gxy, bv, bvg1, bvg2,
                                  b121, b121n, b10m1, b10m1x2,
                                  xsrc_h, xsrc_l, sqx_st, sqy_st,
                                  gxs_sb, gys_sb):
                    """One (image, channel): fp16-pair blur + sobel."""
                    xth = xpool.tile([128, W + 4], f16, tag="xh", name="xth")
                    xtl = xpool.tile([128, W + 4], f16, tag="xl", name="xtl")
                    for xt, src in ((xth, xsrc_h), (xtl, xsrc_l)):
                        nc.gpsimd.memset(xt[:, 0:2], 0.0)
                        nc.gpsimd.memset(xt[:, W + 2:W + 4], 0.0)
                        nc.sync.dma_start(out=xt[0:n_in, 2:W + 2], in_=src)
                    bl = psA.tile([124, W], f32, tag="bl", name="bl")
                    lhs5 = [bv, bvg1, bvg2, bvg1, bv]
                    for lo in (0, 512):
                        first = True
                        for xt in (xth, xtl):
                            for h_ in range(5):
                                nc.tensor.matmul(
                                    out=bl[0:n_bl, lo:lo + 512],
                                    lhsT=lhs5[h_][0:n_in, 0:n_bl],
                                    rhs=xt[0:n_in, h_ + lo:h_ + lo + 512],
                                    start=first,
                                    stop=(xt is xtl and h_ == 4))
                                first = False
                    # fp16 pair of bl for the sobel rhs
                    blh = blspool.tile([124, W + 2], f16, tag="blh",
                                       name="blh")
                    bll = blspool.tile([124, W + 2], f16, tag="bll",
                                       name="bll")
                    for t in (blh, bll):
                        nc.gpsimd.memset(t[:, 0:1], 0.0)
                        nc.gpsimd.memset(t[:, W + 1:W + 2], 0.0)
                    nc.scalar.copy(out=blh[0:n_bl, 1:W + 1], in_=bl[0:n_bl, :])
                    nc.vector.tensor_tensor(out=bll[0:n_bl, 1:W + 1],
                                            in0=bl[0:n_bl, :],
                                            in1=blh[0:n_bl, 1:W + 1],
                                            op=Alu.subtract)
                    gx = psB.tile([122, W], f32, tag="gxy", name="gx")
                    gy = psB.tile([122, W], f32, tag="gxy", name="gy")
                    for lo in (0, 512):
                        for i, blt in enumerate((blh, bll)):
                            st = (i == 0)
                            sp = (i == 1)
                            nc.tensor.matmul(
                                out=gx[0:n_gxy, lo:lo + 512],
                                lhsT=b121[0:n_bl, 0:n_gxy],
                                rhs=blt[0:n_bl, lo:lo + 512],
                                start=st, stop=False)
                            nc.tensor.matmul(
                                out=gx[0:n_gxy, lo:lo + 512],
                                lhsT=b121n[0:n_bl, 0:n_gxy],
                                rhs=blt[0:n_bl, 2 + lo:2 + lo + 512],
                                start=False, stop=sp)
                            nc.tensor.matmul(
                                out=gy[0:n_gxy, lo:lo + 512],
                                lhsT=b10m1[0:n_bl, 0:n_gxy],
                                rhs=blt[0:n_bl, 2 + lo:2 + lo + 512],
                                start=st, stop=False)
                            nc.tensor.matmul(
                                out=gy[0:n_gxy, lo:lo + 512],
                                lhsT=b10m1x2[0:n_bl, 0:n_gxy],
                                rhs=blt[0:n_bl, 1 + lo:1 + lo + 512],
                                start=False, stop=False)
                            nc.tensor.matmul(
                                out=gy[0:n_gxy, lo:lo + 512],
                                lhsT=b10m1[0:n_bl, 0:n_gxy],
                                rhs=blt[0:n_bl, lo:lo + 512],
                                start=False, stop=sp)
                    nc.scalar.square(out=sqx_st[0:n_gxy, c * W:(c + 1) * W],
                                     in_=gx[0:n_gxy, :])
                    nc.scalar.square(out=sqy_st[0:n_gxy, c * W:(c + 1) * W],
                                     in_=gy[0:n_gxy, :])
                    # gxs/gys accumulation in f32 SBUF (masks need f32)
                    if c == 0:
                        nc.scalar.copy(out=gxs_sb[0:n_gxy, :],
                                       in_=gx[0:n_gxy, :])
                        nc.scalar.copy(out=gys_sb[0:n_gxy, :],
                                       in_=gy[0:n_gxy, :])
                    else:
                        nc.vector.tensor_tensor(out=gxs_sb[0:n_gxy, :],
                                                in0=gxs_sb[0:n_gxy, :],
                                                in1=gx[0:n_gxy, :],
                                                op=Alu.add)
                        nc.vector.tensor_tensor(out=gys_sb[0:n_gxy, :],
                                                in0=gys_sb[0:n_gxy, :],
                                                in1=gy[0:n_gxy, :],
                                                op=Alu.add)

                def finish_image(n_gxy, sqx_st, sqy_st, gm_t):
                    """magnitude: m2 (Pool), sqrt (Act), gm chunk adds."""
                    nc.gpsimd.tensor_tensor(out=sqx_st[0:n_gxy, :],
                                            in0=sqx_st[0:n_gxy, :],
                                            in1=sqy_st[0:n_gxy, :],
                                            op=Alu.add)
                    # reuse sqy's buffer: m2 (its last reader) just finished
                    mag = sqpool.tile([122, 3 * W], f32, tag="sqy",
                                      name="mag")
                    nc.scalar.sqrt(out=mag[0:n_gxy, :], in_=sqx_st[0:n_gxy, :])
                    gmi = gm_t[0:n_gxy, 1:W + 1]
                    nc.vector.tensor_tensor(out=gmi, in0=mag[0:n_gxy, 0:W],
                                            in1=mag[0:n_gxy, W:2 * W],
                                            op=Alu.add)
                    nc.vector.tensor_tensor(out=gmi, in0=gmi,
                                            in1=mag[0:n_gxy, 2 * W:3 * W],
                                            op=Alu.add)

                def make_masks(gxs_sb, gys_sb, n, shift, n_thin, j):
                    """u8 class masks at conv frame [0:n], DMA-shifted down
                    by `shift` rows into persistent thin-frame tiles."""
                    a2 = gspool.tile([122, W], f32, tag="a2", name="a2",
                                     bufs=1)
                    b2 = gspool.tile([122, W], f32, tag="b2", name="b2",
                                     bufs=1)
                    nc.scalar.square(out=a2[0:n, :], in_=gxs_sb[0:n, :])
                    nc.scalar.square(out=b2[0:n, :], in_=gys_sb[0:n, :])
                    sgx = gspool.tile([122, W], f16, tag="sgx", name="sgx")
                    sgy = gspool.tile([122, W], f16, tag="sgy", name="sgy")
                    nc.vector.tensor_scalar(out=sgx[0:n, :],
                                            in0=gxs_sb[0:n, :], scalar1=0.0,
                                            scalar2=None, op0=Alu.is_ge)
                    nc.vector.tensor_scalar(out=sgy[0:n, :],
                                            in0=gys_sb[0:n, :], scalar1=0.0,
                                            scalar2=None, op0=Alu.is_ge)
                    tmp = [msktpool.tile([122, W], u8, tag=t, name=t)
                           for t in ("tc0", "tc2", "tsm")]
                    nc.vector.scalar_tensor_tensor(
                        out=tmp[0][0:n, :], in0=a2[0:n, :], scalar=T22SQ,
                        in1=b2[0:n, :], op0=Alu.mult, op1=Alu.is_gt)
                    nc.vector.scalar_tensor_tensor(
                        out=tmp[1][0:n, :], in0=b2[0:n, :], scalar=T22SQ,
                        in1=a2[0:n, :], op0=Alu.mult, op1=Alu.is_gt)
                    # sign agreement == (ab >= 0) wherever c0/c2 don't apply
                    nc.vector.tensor_tensor(out=tmp[2][0:n, :],
                                            in0=sgx[0:n, :], in1=sgy[0:n, :],
                                            op=Alu.is_equal)
                    out = []
                    for t, tag in zip(tmp, ("c0", "c2", "sm")):
                        p = mskpool.tile([n_thin, W], u8, tag=f"{tag}_{j}",
                                         name=f"{tag}_{j}")
                        nc.sync.dma_start(out=p[:],
                                          in_=t[shift:shift + n_thin, :])
                        out.append(p)
                    return out

                # ---- B8 block FIRST so the collectives hide under main conv
                nc.vector.memset(gm8[:], 0.0)
                sqx8 = sqpool.tile([122, 3 * W], f32, tag="sqx", name="sqx")
                sqy8 = sqpool.tile([122, 3 * W], f32, tag="sqy", name="sqy")
                gxs8 = gspool.tile([122, W], f32, tag="gxs", name="gxs")
                gys8 = gspool.tile([122, W], f32, tag="gys", name="gys")
                for c in range(3):
                    conv_pipeline(c, 88, 86, 84, mt["BV_8"], mt["BVG1_8"],
                                  mt["BVG2_8"], mt["B121_8"], mt["B121N_8"],
                                  mt["B10M1_8"], mt["B10M1X2_8"],
                                  x8h[c], x8l[c], sqx8, sqy8, gxs8, gys8)
                finish_image(84, sqx8, sqy8, gm8)
                # thin8 frame = conv rows 3..83 -> shift 3, 81 rows
                m8 = make_masks(gxs8, gys8, 84, 3, 81, 8)

                ag_in = dpool.tile([81, W], f32, tag="ag_in")
                ag_out = dpool.tile([B * 81, W], f32, tag="ag_out")
                nc.sync.dma_start(out=ag_in[:], in_=gm8[3:84, 1:W + 1])
                nc.gpsimd.collective_compute(
                    "AllGather", Alu.bypass,
                    replica_groups=[list(range(NC))],
                    ins=[ag_in.opt()], outs=[ag_out.opt()])
                a2a_in = dpool.tile([B * 81, W], f32, tag="a2a_in")
                a2a_out = dpool.tile([B * 81, W], f32, tag="a2a_out")
                for b in range(B):
                    dr, dc = DELTAS[b]
                    nc.sync.dma_start(
                        out=a2a_in[81 * b:81 * (b + 1), :],
                        in_=gm8[3 + dr:84 + dr, 1 + dc:W + 1 + dc])
                nc.gpsimd.collective_compute(
                    "AllToAll", Alu.bypass,
                    replica_groups=[list(range(NC))],
                    ins=[a2a_in.opt()], outs=[a2a_out.opt()])
                nc.sync.dma_start(out=ce8[:], in_=gm8[3:84, 1:W + 1])

                # ---- main slab: 8 images x 3 channels
                for j in range(B):
                    gm_j = gmpool.tile([122, W + 2], f32, tag=f"gm{j}",
                                       name=f"gm{j}")
                    nc.gpsimd.memset(gm_j[:, 0:1], 0.0)
                    nc.gpsimd.memset(gm_j[:, W + 1:W + 2], 0.0)
                    sqx_st = sqpool.tile([122, 3 * W], f32, tag="sqx",
                                         name="sqx")
                    sqy_st = sqpool.tile([122, 3 * W], f32, tag="sqy",
                                         name="sqy")
                    gxs_sb = gspool.tile([122, W], f32, tag="gxs",
                                         name="gxs")
                    gys_sb = gspool.tile([122, W], f32, tag="gys",
                                         name="gys")
                    for c in range(3):
                        conv_pipeline(c, 128, 124, 122, mt["BV"], mt["BVG1"],
                                      mt["BVG2"], mt["B121"], mt["B121N"],
                                      mt["B10M1"], mt["B10M1X2"],
                                      xh[3 * j + c], xl[3 * j + c],
                                      sqx_st, sqy_st, gxs_sb, gys_sb)
                    finish_image(122, sqx_st, sqy_st, gm_j)
                    gm_tiles.append(gm_j)
                    # dirs 0-4 (shifted frame): masks shift 0, 121 rows;
                    # dirs 5-7 (thin frame): masks shift 1, 120 rows.
                    if j <= 4:
                        masks.append(make_masks(gxs_sb, gys_sb, 122, 0,
                                                121, j))
                    else:
                        masks.append(make_masks(gxs_sb, gys_sb, 122, 1,
                                                120, j))
                    cet = gmpool.tile([121, W + 2], f32, tag=f"ce{j}",
                                      name=f"ce{j}")
                    nc.sync.dma_start(out=cet[:], in_=gm_j[1:122, :])
                    ce.append(cet)

            # =========== NMS phase ========================================
            # thin frame (dirs 5-7): row p <-> img 118i-1+p, 120 rows.
            # shifted frame (dirs 0-4): row r <-> thin p=r-1, 121 rows.
            with (
                tc.tile_pool(name="cmap", bufs=4) as cpool2,
                tc.tile_pool(name="pmap", bufs=1) as ppool,
                tc.tile_pool(name="g8p", bufs=3) as g8pool,
                tc.tile_pool(name="nmst", bufs=2) as npool,
                tc.tile_pool(name="outp", bufs=2) as opool,
                tc.tile_pool(name="psC", bufs=2, space="PSUM") as psC,
            ):
                def nms_core(b_masks, gm_b, get_in0, get_in1, n_thin,
                             bones, n_mp, out_lo, out_dram, n_out):
                    """Shared NMS tail; all tiles base partition 0.
                    n_thin = working rows (121 shifted / 120 thin / 81 B8).
                    gm_b = f32 AP for thresholds, n_thin rows."""
                    c0, c2, sm = b_masks
                    P = []
                    for k in range(4):
                        Cs = []
                        for j in (k, k + 4):
                            cj = cpool2.tile([121, W], f16, tag="c", name="c")
                            nc.vector.tensor_tensor(
                                out=cj[0:n_thin, :], in0=get_in0(j),
                                in1=get_in1(j), op=Alu.is_gt)
                            Cs.append(cj)
                        tag = "psel" if k == 3 else f"p{k}"
                        pk = ppool.tile([121, W], f16, tag=tag, name=tag,
                                        bufs=2)
                        if k % 2:
                            nc.vector.tensor_tensor(
                                out=pk[0:n_thin, :], in0=Cs[0][0:n_thin, :],
                                in1=Cs[1][0:n_thin, :], op=Alu.logical_and)
                        else:
                            # and of {0,1} masks == product (Pool has no
                            # logical ops)
                            nc.gpsimd.tensor_tensor(
                                out=pk[0:n_thin, :], in0=Cs[0][0:n_thin, :],
                                in1=Cs[1][0:n_thin, :], op=Alu.mult)
                        P.append(pk)
                    psel = P[3]
                    nc.vector.copy_predicated(out=psel[0:n_thin, :],
                                              mask=sm[0:n_thin, :],
                                              data=P[1][0:n_thin, :])
                    nc.vector.copy_predicated(out=psel[0:n_thin, :],
                                              mask=c0[0:n_thin, :],
                                              data=P[0][0:n_thin, :])
                    nc.vector.copy_predicated(out=psel[0:n_thin, :],
                                              mask=c2[0:n_thin, :],
                                              data=P[2][0:n_thin, :])
                    tq = npool.tile([121, W], f16, tag="tq", name="tq")
                    th = npool.tile([121, W], f16, tag="th", name="th")
                    nc.vector.tensor_scalar(out=tq[0:n_thin, :], in0=gm_b,
                                            scalar1=LOW_S, scalar2=None,
                                            op0=Alu.is_ge)
                    nc.vector.tensor_scalar(out=th[0:n_thin, :], in0=gm_b,
                                            scalar1=HIGH_S, scalar2=None,
                                            op0=Alu.is_gt)
                    q = npool.tile([121, W], f16, tag="q", name="q")
                    nc.vector.tensor_tensor(out=q[0:n_thin, :],
                                            in0=tq[0:n_thin, :],
                                            in1=psel[0:n_thin, :],
                                            op=Alu.logical_and)
                    strong = npool.tile([121, W + 2], f16, tag="strong",
                                        name="strong")
                    nc.gpsimd.memset(strong[:, 0:1], 0.0)
                    nc.gpsimd.memset(strong[:, W + 1:W + 2], 0.0)
                    nc.vector.tensor_tensor(out=strong[0:n_thin, 1:W + 1],
                                            in0=th[0:n_thin, :],
                                            in1=q[0:n_thin, :],
                                            op=Alu.logical_and)
                    # mp = 3x3 box sum of strong: 3 col-shifted matmuls
                    mp = psC.tile([120, W], f32, tag="mp", name="mp")
                    for lo2 in (0, 512):
                        for t in range(3):
                            nc.tensor.matmul(
                                out=mp[0:n_mp, lo2:lo2 + 512],
                                lhsT=bones[0:n_thin, 0:n_mp],
                                rhs=strong[0:n_thin, t + lo2:t + lo2 + 512],
                                start=(t == 0), stop=(t == 2))
                    ot = opool.tile([120, W], f16, tag="ot", name="ot")
                    nc.vector.scalar_tensor_tensor(
                        out=ot[0:n_mp, :], in0=mp[0:n_mp, :], scalar=0.5,
                        in1=q[0:n_mp, :], op0=Alu.is_ge, op1=Alu.logical_and)
                    nc.gpsimd.memset(ot[0:n_mp, 0:1], 0.0)
                    nc.gpsimd.memset(ot[0:n_mp, W - 1:W], 0.0)
                    nc.sync.dma_start(out=out_dram,
                                      in_=ot[out_lo:out_lo + n_out, :])

                def nms_b(b):
                    dr, dc = DELTAS[b]
                    if b <= 4:
                        # shifted frame, 121 rows: in0 = raw gm rows 0..120,
                        # +1-row shift = ce (gm rows 1..121)
                        def in0(j):
                            return gm_tiles[j][0:121, 1:W + 1]

                        def in1(j):
                            if dr == 0:
                                return gm_tiles[j][0:121, 1 + dc:W + 1 + dc]
                            return ce[j][0:121, 1 + dc:W + 1 + dc]

                        nms_core(masks[b], gm_tiles[b][0:121, 1:W + 1],
                                 in0, in1, 121, mt["BONES_D"], 120, 2,
                                 outm[b], SLAB)
                    else:
                        # thin frame, 120 rows: in0 = ce rows 0..119
                        # (gm rows 1..120), -1-row shift = raw gm rows 0..119
                        def in0(j):
                            return ce[j][0:120, 1:W + 1]

                        def in1(j):
                            return gm_tiles[j][0:120, 1 + dc:W + 1 + dc]

                        nms_core(masks[b], ce[b][0:120, 1:W + 1],
                                 in0, in1, 120, mt["BONES"], 119, 1,
                                 outm[b], SLAB)

                for b in (0, 4, 1, 2, 3, 5, 6, 7):
                    nms_b(b)

                # B8: own image only; shifted operands came via AllToAll.
                # thin8 frame: row p (base 0) <-> img 943+p, 81 rows.
                def load8(dram_src, tag):
                    def get(j):
                        t = g8pool.tile([81, W], f32, tag=tag, name=tag,
                                        bufs=3)
                        nc.sync.dma_start(
                            out=t[:], in_=dram_src[81 * j:81 * (j + 1), :])
                        return t[:]
                    return get

                nms_core(m8, ce8[:], load8(ag_out, "g8p"),
                         load8(a2a_out, "g8s"), 81,
                         mt["BONES8"], 81, 1, out8[:], B8_ROWS)

    _legalize_waits(nc)
    _CACHE["nc"] = nc
    return nc


def _legalize_waits(nc):
    """Several ISA encodings (S2S2D2_STT, HWDGE DMACopy, ...) hold only one
    embedded sync-wait, but Tile's scheduler can attach more. Hoist all
    embedded waits of multi-wait instructions into a NoOp injected just
    before them on the same engine queue (NoOps carry many waits fine)."""
    import concourse.mybir as mybir
    n = 0
    for f in nc.m.functions:
        for blk in f.blocks:
            out = []
            for ins in blk.instructions:
                si = ins.sync_info
                if (si is not None and si.on_wait is not None
                        and len(si.on_wait) > 1):
                    for w in si.on_wait:
                        nop = mybir.InstNoOp(
                            name=f"WFIX-{n}", engine=ins.engine,
                            sync_info=mybir.SyncInfo(on_wait=[w],
                                                     on_update=[]))
                        n += 1
                        out.append(nop)
                    ins.sync_info = mybir.SyncInfo(
                        on_wait=[],
                        on_update=list(si.on_update or []))
                out.append(ins)
            blk.instructions = out


def _in_maps(img):
    img = np.asarray(img, dtype=np.float32)
    hi = img.astype(np.float16)
    lo = (img - hi.astype(np.float32)).astype(np.float16)
    pad = np.zeros((B, 3, 5, W), np.float16)
    hip = np.concatenate([pad, hi], axis=2)  # rows shifted by +5
    lop = np.concatenate([pad, lo], axis=2)
    maps = []
    for i in range(NC):
        r0 = SLAB * i  # padded row index of img row 118i-5
        m = {"xh": np.ascontiguousarray(
                 hip[:, :, r0:r0 + 128, :].reshape(B * 3, 128, W)),
             "xl": np.ascontiguousarray(
                 lop[:, :, r0:r0 + 128, :].reshape(B * 3, 128, W)),
             "x8h": np.ascontiguousarray(hi[i, :, B8_START - 8:, :]),
             "x8l": np.ascontiguousarray(lo[i, :, B8_START - 8:, :])}
        m.update(_const_mats(i))
        maps.append(m)
    return maps


def kernel(img, gauss_h=None, gauss_v=None, sobel_h=None, sobel_v=None,
           dir_f=None, connect_f=None, _want_time=False):
    from concourse.bass_utils import run_bass_kernel_spmd
    nc = _build_program()
    maps = _in_maps(np.asarray(img))
    res = run_bass_kernel_spmd(nc, maps, list(range(NC)), trace=_want_time)
    out = np.zeros((B, 1, H, W), np.float32)
    for i in range(NC):
        r = res.results[i]
        out[:, 0, SLAB * i:SLAB * (i + 1), :] = \
            np.asarray(r["outm"], np.float32)
        out[i, 0, B8_START:, :] = np.asarray(r["out8"], np.float32)
    if _want_time:
        return out, res
    return out
